# revision 1
# baseline (speedup 1.0000x reference)
"""Trainium2 Bass kernel for AdaptiveSpectralFeatureRefinementCosine (v2).

Math (per batch, pixel x, 3x3 window taps k, C=128 channels):
    d_k(x)  = <fe(:,x), fp(:,x+dk)>                (raw dot, bf16 products)
    cos_k   = d_k * re(x) * rf(x+dk)               (re=1/||fe||, rf=1/||fp||)
    e_k     = exp(cos_k)
    w_k     = e_k * nf(x+dk) / S(x),  S = sum_k e_k,  nf = ||fp||
    out     = sum_k [e_k/S] * fp_k + fe

Layouts:
  - channel-major [C=128 part, pixels free] for products / weighted taps
  - tap-row-major [72 = 9 taps x 8 rows part, W=128 free] for the softmax
    mid-section (partition p = 8*k + r); selector matmuls (sliding-band and
    block-identity lhsT consts) move data between the two layouts, since PE
    operands must sit at base partition 0/32/64.
  - per-tap weighted taps accumulate into PSUM via identity matmuls.

Sharding: B*H = 512 image rows -> 64 rows per core on 8 cores
(core = 2*b + rowhalf). Device gets fe slab (C,64,128) and zero-padded
fused slab (C,66,130) incl. halo -> no edge handling on device.
"""
import sys

sys.path.insert(0, "/opt/trn_rl_repo")
import numpy as np

B, C, H, W = 4, 128, 128, 128
ROWS = 64                   # output rows per core
FR, FC = ROWS + 2, W + 2    # fused slab (66, 130)
BR = 8                      # rows per block
NBLK = ROWS // BR           # 8 blocks
TR = 9 * BR                 # tap-row partitions (72)
HB = 4                      # rows per aggregation half-block

_CACHE = {}


def _build_nc(reps=1):
    from concourse import bass, tile, bacc

    mybir = bass.mybir
    F32 = mybir.dt.float32
    BF16 = mybir.dt.bfloat16
    MUL = mybir.AluOpType.mult
    ADD = mybir.AluOpType.add
    AF = mybir.ActivationFunctionType

    TAPS = [(di, dj) for di in range(3) for dj in range(3)]

    nc = bacc.Bacc(None, target_bir_lowering=False)
    fe_ext = nc.declare_dram_parameter("fe", [C, ROWS, W], F32, isOutput=False)
    fp_ext = nc.declare_dram_parameter("fp", [C, FR, FC], F32, isOutput=False)
    out_ext = nc.declare_dram_parameter("out", [C, ROWS, W], F32, isOutput=True)

    with tile.TileContext(nc) as tc:
        with (
            tc.tile_pool(name="big", bufs=1) as big,
            tc.tile_pool(name="cst", bufs=1) as cst,
            tc.tile_pool(name="wk", bufs=3) as wk,
            tc.tile_pool(name="sm", bufs=3) as sm,
            tc.tile_pool(name="gkp", bufs=3) as gkp,
            tc.tile_pool(name="psA", bufs=2, space="PSUM") as psA,
            tc.tile_pool(name="psV", bufs=2, space="PSUM") as psV,
            tc.tile_pool(name="psU", bufs=2, space="PSUM") as psU,
        ):
            # ---------------- persistent tiles ----------------
            fe_sb = big.tile([C, ROWS, W], F32)
            fp_sb = big.tile([C, FR, FC], F32)
            fe_bf = big.tile([C, ROWS, W], BF16)
            fp_bf = big.tile([C, FR, FC], BF16)
            re_bf = big.tile([ROWS, W], BF16)     # 1/||fe||
            rf_bf = big.tile([FR, FC], BF16)      # 1/||fp||
            # block-major re-based norm tables: [r, ib, col]
            rfB = [big.tile([8, NBLK, FC], BF16, name=f"rfB{d}")
                   for d in range(3)]
            reB = big.tile([8, NBLK, W], BF16)
            rrf_all = big.tile([TR, NBLK, W], BF16)   # re*rf shift table

            id8 = cst.tile([8, 8], BF16)          # identity block
            idc = cst.tile([C, C], BF16)          # identity for PSUM adds
            sel_s = cst.tile([TR, 8], BF16)       # col r -> ones at p=8k+r
            sel_b = cst.tile([8, TR], BF16)       # col 8k+r -> one at row r
            band_tr = cst.tile([C, 2 * TR - 1], BF16)   # ones col at TR-1
            band_ne = cst.tile([C, 2 * ROWS - 1], BF16)
            band_nf = cst.tile([C, 2 * FR - 1], BF16)
            bnd2 = cst.tile([8, 2 * TR], BF16)    # 1 at (j, TR-8+j)
            bid = cst.tile([FR, FR + 8], BF16)    # 1 at (p, p+7)
            esel = cst.tile([TR, TR * C], BF16)   # row p ones in [pC,(p+1)C)

            # ---------------- input DMA ----------------
            for ch in range(8):
                a, b2 = 8 * ch, 8 * (ch + 1)
                nc.sync.dma_start(fe_sb[:, a:b2, :], fe_ext[:, a:b2, :])
            for ch in range(6):
                a = 11 * ch
                b2 = min(FR, 11 * (ch + 1))
                nc.sync.dma_start(fp_sb[:, a:b2, :], fp_ext[:, a:b2, :])

            # ---------------- constants ----------------
            EQ = mybir.AluOpType.is_equal
            GE = mybir.AluOpType.is_ge
            LE = mybir.AluOpType.is_le

            def diag_const(t, ncols, dshift):
                # t[p, j] = 1 iff j == p + dshift
                nc.vector.memset(t, 1.0)
                nc.gpsimd.affine_select(t, t, [[1, ncols]], EQ, 0.0,
                                        base=-dshift, channel_multiplier=-1)

            def col_const(t, ncols, col):
                # t[p, j] = 1 iff j == col (all partitions)
                nc.vector.memset(t, 1.0)
                nc.gpsimd.affine_select(t, t, [[1, ncols]], EQ, 0.0,
                                        base=-col, channel_multiplier=0)

            diag_const(id8[:], 8, 0)
            diag_const(idc[:], C, 0)
            diag_const(bnd2[:], 2 * TR, TR - 8)
            diag_const(bid[:], FR + 8, 7)
            col_const(band_tr[:], 2 * TR - 1, TR - 1)
            col_const(band_ne[:], 2 * ROWS - 1, ROWS - 1)
            col_const(band_nf[:], 2 * FR - 1, FR - 1)
            nc.vector.memset(sel_s[:], 0.0)
            nc.vector.memset(sel_b[:], 0.0)
            # esel[p, j] = 1 iff C*p <= j < C*(p+1)
            nc.gpsimd.memset(esel[:], 1.0)
            nc.gpsimd.affine_select(esel[:], esel[:], [[1, TR * C]], GE, 0.0,
                                    base=0, channel_multiplier=-C)
            nc.gpsimd.affine_select(esel[:], esel[:], [[-1, TR * C]], GE,
                                    0.0, base=C - 1, channel_multiplier=C)
            for k in range(9):
                nc.sync.dma_start(sel_s[8 * k:8 * k + 8, :], id8[:])
                nc.sync.dma_start(sel_b[:, 8 * k:8 * k + 8], id8[:])
            for b8 in range(8):
                nc.sync.dma_start(
                    bid[8 * b8:8 * b8 + 8, 7 + 8 * b8:15 + 8 * b8], id8[:])
            nc.sync.dma_start(bid[64:66, 71:73], id8[0:2, 0:2])

            # ---------------- casts (Act) ----------------
            for ch in range(4):
                a, b2 = 16 * ch, 16 * (ch + 1)
                nc.scalar.copy(fe_bf[:, a:b2, :], fe_sb[:, a:b2, :])
            for ch in range(6):
                a = 11 * ch
                b2 = min(FR, 11 * (ch + 1))
                nc.scalar.copy(fp_bf[:, a:b2, :], fp_sb[:, a:b2, :])

            # ---------------- norms ----------------
            ne2t = psU.tile([TR, 384], F32, tag="U")
            ne2 = ne2t[0:ROWS, 0:W]
            for ch in range(8):
                a = 8 * ch
                sq = wk.tile([C, 8, W], BF16, tag="sqe")
                nc.gpsimd.tensor_tensor(sq[:], fe_bf[:, a:a + 8, :],
                                        fe_bf[:, a:a + 8, :], MUL)
                for r in range(8):
                    y = a + r
                    nc.tensor.matmul(
                        ne2, band_ne[:, ROWS - 1 - y:2 * ROWS - 1 - y],
                        sq[:, r, :], start=(y == 0), stop=(y == ROWS - 1))
            nem = wk.tile([ROWS, W], F32, tag="nem")
            nc.vector.tensor_scalar_max(nem[:], ne2, 1e-24)
            ne_f = wk.tile([ROWS, W], F32, tag="nef")
            nc.scalar.activation(ne_f[:], nem[:], AF.Sqrt)
            with nc.allow_low_precision(reason="bf16 1/norm within budget"):
                nc.vector.reciprocal(re_bf[:], ne_f[:])

            nf2t = psU.tile([TR, 384], F32, tag="U")
            nf2 = nf2t[0:FR, 0:FC]
            for ch in range(9):
                a = 8 * ch
                b2 = min(FR, a + 8)
                sq = wk.tile([C, b2 - a, FC], BF16, tag="sqf")
                nc.gpsimd.tensor_tensor(sq[:], fp_bf[:, a:b2, :],
                                        fp_bf[:, a:b2, :], MUL)
                for r in range(b2 - a):
                    y = a + r
                    nc.tensor.matmul(
                        nf2, band_nf[:, FR - 1 - y:2 * FR - 1 - y],
                        sq[:, r, :], start=(y == 0), stop=(y == FR - 1))
            nfm = wk.tile([FR, FC], F32, tag="nfm")
            nc.vector.tensor_scalar_max(nfm[:], nf2, 1e-24)
            nf_f = wk.tile([FR, FC], F32, tag="nff")
            nc.scalar.activation(nf_f[:], nfm[:], AF.Sqrt)
            with nc.allow_low_precision(reason="bf16 1/norm within budget"):
                nc.vector.reciprocal(rf_bf[:], nf_f[:])

            # --- re-base rf/nf/re into block-major [r, ib, col] tiles so
            # per-block selector matmuls get base-partition-0 operands ---
            def rebase(dst, src_full, off, ncols):
                nsrc = src_full.shape[0]
                for ibp in range(4):     # 2 blocks per psum chunk
                    bp = psU.tile([TR, 384], F32, tag="U")
                    for q in range(2):
                        ib = 2 * ibp + q
                        s = 8 * ib + off + 7
                        nc.tensor.matmul(
                            bp[0:8, ncols * q:ncols * (q + 1)],
                            bid[0:nsrc, s:s + 8], src_full,
                            start=True, stop=True)
                    nc.scalar.copy(
                        dst[:, 2 * ibp:2 * ibp + 2, :].rearrange(
                            "r ib u -> r (ib u)"),
                        bp[0:8, 0:2 * ncols])

            for di in range(3):
                rebase(rfB[di], rf_bf[:], di, FC)
            rebase(reB, re_bf[:], 0, W)

            # --- precompute shift tables [72, ib, 128] (p = 8k + r) ---
            for ib in range(NBLK):
                T = psU.tile([TR, 384], F32, tag="U")
                rfp, rep = T[:, 0:128], T[:, 128:256]
                for k, (di, dj) in enumerate(TAPS):
                    sel = bnd2[:, TR - 8 - 8 * k:2 * TR - 8 - 8 * k]
                    nc.tensor.matmul(rfp, sel, rfB[di][:, ib, dj:dj + W],
                                     start=(k == 0), stop=(k == 8))
                    nc.tensor.matmul(rep, sel, reB[:, ib, :],
                                     start=(k == 0), stop=(k == 8))
                re9sb = sm.tile([TR, W], BF16, tag="re9sb")
                nc.scalar.copy(re9sb[:], rep)
                nc.vector.tensor_tensor(rrf_all[:, ib, :], rfp, re9sb[:], MUL)

            # ---------------- compute (repeated for timing) ----------------
            import os
            from contextlib import nullcontext
            pyloop = os.environ.get("BASS_PYLOOP", "0") == "1"
            with (tc.For_i(0, reps, 1) if reps > 1 and not pyloop
                  else nullcontext()):
             for _rep in range(reps if pyloop else 1):
              for ib in range(NBLK):
                i0 = BR * ib

                rrf = rrf_all[:, ib, :]
                T = psU.tile([TR, 384], F32, tag="U")
                cosp = T[:, 0:128]

                # ---- products + cos matmuls ----
                for k, (di, dj) in enumerate(TAPS):
                    pr = wk.tile([C, BR, W], BF16, tag=f"pr{k % 2}")
                    peng = nc.gpsimd if k in (1, 3, 5, 7) else nc.vector
                    peng.tensor_tensor(
                        pr[:], fe_bf[:, i0:i0 + BR, :],
                        fp_bf[:, i0 + di:i0 + di + BR, dj:dj + W], MUL)
                    for r in range(BR):
                        p = 8 * k + r
                        nc.tensor.matmul(
                            cosp, band_tr[:, TR - 1 - p:2 * TR - 1 - p],
                            pr[:, r, :], start=(p == 0), stop=(p == TR - 1))

                # ---- softmax section ----
                cosn = sm.tile([TR, W], F32, tag="cosn")
                nc.vector.tensor_tensor(cosn[:], cosp, rrf, MUL)
                wexp = sm.tile([TR, W], BF16, tag="wexp")
                nc.scalar.activation(wexp[:], cosn[:], AF.Exp)
                sp, rcp9 = T[0:8, 128:256], T[:, 256:384]
                nc.tensor.matmul(sp, sel_s[:], wexp[:], start=True, stop=True)
                rr = sm.tile([8, W], BF16, tag="rr")
                with nc.allow_low_precision(reason="bf16 1/S within budget"):
                    nc.vector.reciprocal(rr[:], sp)
                nc.tensor.matmul(rcp9, sel_b[:], rr[:], start=True, stop=True)
                w2 = sm.tile([TR, W], BF16, tag="w2")
                nc.vector.tensor_tensor(w2[:], wexp[:], rcp9, MUL)

                # ---- aggregation per 4-row half ----
                # full-block vb/vbs/gk; acc stays per-half (PSUM banks)
                accs = [psA.tile([C, HB, W], F32, tag="acc", name=f"acc{h}")
                        for h in range(2)]

                def emit_vb(k):
                    vb = psV.tile([C, BR, W], F32, tag="vb", name=f"vb{k}")
                    for r in range(BR):
                        p = 8 * k + r
                        nc.tensor.matmul(
                            vb[:, r, :], esel[:, C * p:C * (p + 1)],
                            w2[:], start=True, stop=True)
                    return vb

                def emit_gk(t, vb):
                    di, dj = TAPS[t]
                    k = t
                    gk = gkp.tile([C, BR, W], BF16, tag=f"gk{k % 3}",
                                  name=f"gk{k}")
                    vbs = gkp.tile([C, BR, W], BF16, tag=f"vbs{k % 2}",
                                   name=f"vbs{k}")
                    nc.scalar.copy(vbs[:], vb[:])
                    nc.vector.tensor_tensor(
                        gk[:],
                        fp_bf[:, i0 + di:i0 + di + BR, dj:dj + W],
                        vbs[:], MUL)
                    return gk

                def emit_acc(gk, gi, ng):
                    for h in range(2):
                        nc.tensor.matmul(
                            accs[h][:].rearrange("c r x -> c (r x)"), idc[:],
                            gk[:, HB * h:HB * h + HB, :].rearrange(
                                "c r x -> c (r x)"),
                            start=(gi == 0), stop=(gi == ng - 1))

                pend = {}
                # groups: (2+3) DVE-added opens the chain; 4..8 singles;
                # (0+1) Pool-added closes it (Pool add runs off-path)
                def handle(t, vb):
                    gk = emit_gk(t, vb)
                    if t == 0:
                        pend[0] = gk
                    elif t == 1:
                        gsum_p = gkp.tile([C, BR, W], BF16, tag="gs")
                        nc.gpsimd.tensor_tensor(gsum_p[:], pend.pop(0)[:],
                                                gk[:], ADD)
                        pend["p"] = gsum_p
                    elif t == 2:
                        pend[2] = gk
                    elif t == 3:
                        gsum_v = gkp.tile([C, BR, W], BF16, tag="gs")
                        nc.vector.tensor_tensor(gsum_v[:], pend.pop(2)[:],
                                                gk[:], ADD)
                        emit_acc(gsum_v, 0, 7)
                    else:
                        emit_acc(gk, t - 3, 7)
                        if t == 8:
                            emit_acc(pend.pop("p"), 6, 7)

                vb_prev, tprev = emit_vb(0), 0
                for t in range(1, 9):
                    vb_next = emit_vb(t)
                    handle(tprev, vb_prev)
                    vb_prev, tprev = vb_next, t
                handle(tprev, vb_prev)
                for h in range(2):
                    r0 = HB * h
                    ot = gkp.tile([C, HB, W], F32, tag="ot")
                    nc.vector.tensor_tensor(
                        ot[:], accs[h][:],
                        fe_sb[:, i0 + r0:i0 + r0 + HB, :], ADD)
                    nc.sync.dma_start(out_ext[:, i0 + r0:i0 + r0 + HB, :],
                                      ot[:])
    nc.finalize()
    return nc


def _get_nc(reps=1):
    key = f"nc{reps}"
    if key not in _CACHE:
        _CACHE[key] = _build_nc(reps)
    return _CACHE[key]


def _shard_inputs(fe_lv, fused_features):
    fe_lv = np.ascontiguousarray(fe_lv, dtype=np.float32)
    fp = np.zeros((B, C, H + 2, W + 2), dtype=np.float32)
    fp[:, :, 1:-1, 1:-1] = fused_features
    in_maps = []
    for core in range(8):
        b, half = core // 2, core % 2
        r0 = half * ROWS
        in_maps.append({
            "fe": np.ascontiguousarray(fe_lv[b, :, r0:r0 + ROWS, :]),
            "fp": np.ascontiguousarray(fp[b, :, r0:r0 + FR, :]),
        })
    return in_maps


def kernel(fe_lv, fused_features):
    from concourse.bass_utils import run_bass_kernel_spmd

    nc = _get_nc()
    in_maps = _shard_inputs(fe_lv, fused_features)
    res = run_bass_kernel_spmd(nc, in_maps, core_ids=list(range(8)))
    out = np.empty((B, C, H, W), dtype=np.float32)
    for core in range(8):
        b, half = core // 2, core % 2
        out[b, :, half * ROWS:half * ROWS + ROWS, :] = res.results[core]["out"]
    return out

